# revision 1
# baseline (speedup 1.0000x reference)
"""BiDAF attention kernel v14: software-pipelined — batch b's c2qT chains
interleave into batch b+1's sim/exp slots at jt granularity, smoothing ACT
and DVE load across the whole invocation. Math identical to v7/v9.
"""

import numpy as np

B, LC, LQ, D = 16, 1024, 1024, 256
N_CORES = 8
BPC = B // N_CORES
NJ = LQ // 128
NI = LC // 128

_CACHE = {}


def build_program(repeat_inner=1, n_cores=N_CORES, ablate=(), loop_n=None):
    import concourse.bacc as bacc
    import concourse.tile as tile
    from concourse import mybir
    from contextlib import nullcontext

    f32 = mybir.dt.float32
    bf16 = mybir.dt.bfloat16

    nc = bacc.Bacc(
        "TRN2",
        target_bir_lowering=False,
        debug=False,
        enable_asserts=False,
        num_devices=n_cores,
    )

    kT_d = nc.dram_tensor("kt", [BPC, D, LQ], bf16, kind="ExternalInput").ap()
    cT_d = nc.dram_tensor("ct", [BPC, D, LC], bf16, kind="ExternalInput").ap()
    qa_d = nc.dram_tensor("qa", [BPC, LQ, D], bf16, kind="ExternalInput").ap()
    qb_d = nc.dram_tensor("qb", [BPC, 128, NJ], f32, kind="ExternalInput").ap()

    cqT_d = nc.dram_tensor(
        "cqt", [BPC, 2, 128, LC], bf16, kind="ExternalOutput").ap()
    red_d = nc.dram_tensor(
        "red", [BPC, 128, 2, NI], f32, kind="ExternalOutput").ap()

    CHAINS = [(db, nh) for db in range(2) for nh in range(2)]

    with tile.TileContext(nc) as tc:
        with (
            tc.tile_pool(name="io", bufs=3) as io_pool,
            tc.tile_pool(name="et", bufs=2) as et_pool,
            tc.tile_pool(name="tree", bufs=2) as tree_pool,
            tc.tile_pool(name="outs", bufs=2) as out_pool,
            tc.tile_pool(name="psum_sim", bufs=2, space="PSUM") as sim_pool,
            tc.tile_pool(name="psum_c2q", bufs=1, space="PSUM") as c2q_pool,
        ):
            loop_cm = tc.For_i(0, loop_n, 1) if loop_n is not None else nullcontext()
            with loop_cm:
                prev = [None]   # (b, ET, qa_s, pcs) of the previous batch

                def emit_c2q_matmuls(k):
                    """jc=k matmul of all 4 chains of the previous batch."""
                    if prev[0] is None or "c2q" in ablate:
                        return
                    _pb, pET, pqa, pcs = prev[0]
                    for ci, (db, nh) in enumerate(CHAINS):
                        nc.tensor.matmul(
                            pcs[ci][:],
                            lhsT=pqa[:, k, db * 128:(db + 1) * 128],
                            rhs=pET[:, k, nh * 512:(nh + 1) * 512],
                            start=(k == 0), stop=(k == NJ - 1),
                        )

                def emit_c2q_tail():
                    """copies + output DMA for the previous batch."""
                    if prev[0] is None or "c2q" in ablate:
                        return
                    pb, _pET, _pqa, pcs = prev[0]
                    cqo = out_pool.tile([128, 2, LC], bf16, tag="cqo")
                    for ci, (db, nh) in enumerate(CHAINS):
                        nc.vector.tensor_copy(
                            cqo[:, db, nh * 512:(nh + 1) * 512], pcs[ci][:])
                    nc.scalar.dma_start(
                        cqT_d[pb].rearrange("a p n -> p a n", p=128), cqo[:])
                    prev[0] = None

                def emit_batch(b, pipelined):
                    kT_s = io_pool.tile([128, 2, LQ], bf16, tag="kt")
                    cT_s = io_pool.tile([128, 2, LC], bf16, tag="ct")
                    kT_r = kT_d[b].rearrange("(c p) n -> p c n", p=128)
                    cT_r = cT_d[b].rearrange("(c p) n -> p c n", p=128)
                    nc.sync.dma_start(kT_s[:, :, 0:128], kT_r[:, :, 0:128])
                    nc.sync.dma_start(cT_s[:, :, 0:512], cT_r[:, :, 0:512])
                    qb_s = io_pool.tile([128, NJ], f32, tag="qb")
                    nc.sync.dma_start(qb_s[:], qb_d[b])
                    nc.sync.dma_start(cT_s[:, :, 512:LC], cT_r[:, :, 512:LC])
                    nc.sync.dma_start(kT_s[:, :, 128:LQ], kT_r[:, :, 128:LQ])
                    qa_s = io_pool.tile([128, NJ, D], bf16, tag="qa")
                    nc.sync.dma_start(
                        qa_s[:], qa_d[b].rearrange("(c p) n -> p c n", p=128))

                    ET = et_pool.tile([128, NJ, LC], bf16, tag="et")
                    t1m = tree_pool.tile([128, 4, LC], bf16, tag="t1m")
                    t1s = tree_pool.tile([128, 4, LC], bf16, tag="t1s")
                    t2m = tree_pool.tile([128, 2, LC], bf16, tag="t2m")
                    t2s = tree_pool.tile([128, 2, LC], bf16, tag="t2s")
                    ms = tree_pool.tile([128, 2, LC], bf16, tag="ms")
                    for jt in range(NJ):
                        ps = sim_pool.tile([128, LC], f32, tag="sim")
                        if "sim" not in ablate:
                            for nh in range(2):
                                cols = slice(nh * 512, (nh + 1) * 512)
                                for dc in range(2):
                                    nc.tensor.matmul(
                                        ps[:, cols],
                                        lhsT=kT_s[:, dc,
                                                  jt * 128:(jt + 1) * 128],
                                        rhs=cT_s[:, dc, cols],
                                        start=(dc == 0), stop=(dc == 1),
                                    )
                        if "exp" not in ablate:
                            nc.scalar.activation(
                                ET[:, jt, :], ps[:],
                                mybir.ActivationFunctionType.Exp,
                                bias=qb_s[:, jt:jt + 1], scale=1.0,
                            )
                        if pipelined:
                            emit_c2q_matmuls(jt)
                        if "max" not in ablate and jt % 2 == 1:
                            u = jt // 2
                            nc.vector.tensor_max(
                                t1m[:, u, :], ET[:, 2 * u, :],
                                ET[:, 2 * u + 1, :])
                            nc.vector.tensor_add(
                                t1s[:, u, :], ET[:, 2 * u, :],
                                ET[:, 2 * u + 1, :])
                            if jt == 3:
                                nc.vector.tensor_max(
                                    t2m[:, 0, :], t1m[:, 0, :], t1m[:, 1, :])
                                nc.vector.tensor_add(
                                    t2s[:, 0, :], t1s[:, 0, :], t1s[:, 1, :])
                            if jt == 7:
                                nc.vector.tensor_max(
                                    t2m[:, 1, :], t1m[:, 2, :], t1m[:, 3, :])
                                nc.vector.tensor_add(
                                    t2s[:, 1, :], t1s[:, 2, :], t1s[:, 3, :])
                                nc.vector.tensor_max(
                                    ms[:, 0, :], t2m[:, 0, :], t2m[:, 1, :])
                                nc.vector.tensor_add(
                                    ms[:, 1, :], t2s[:, 0, :], t2s[:, 1, :])

                    if pipelined:
                        emit_c2q_tail()

                    if "max" not in ablate:
                        msT = tree_pool.tile([128, 2 * NI, 128], bf16, tag="msT")
                        nc.scalar.dma_start_transpose(
                            msT[:], ms[:].rearrange("p a n -> p (a n)"))
                        red_s = out_pool.tile([128, 2, NI, 1], f32, tag="red")
                        nc.vector.reduce_max(
                            out=red_s[:, 0], in_=msT[:, 0:NI, :],
                            axis=mybir.AxisListType.X)
                        nc.vector.reduce_sum(
                            out=red_s[:, 1], in_=msT[:, NI:2 * NI, :],
                            axis=mybir.AxisListType.X)
                        nc.scalar.dma_start(red_d[b], red_s[:, :, :, 0])

                    pc0 = c2q_pool.tile([128, 512], f32, tag="c0")
                    pc1 = c2q_pool.tile([128, 512], f32, tag="c1")
                    pc2 = c2q_pool.tile([128, 512], f32, tag="c2")
                    pc3 = c2q_pool.tile([128, 512], f32, tag="c3")
                    prev[0] = (b, ET, qa_s, [pc0, pc1, pc2, pc3])

                if loop_n is not None:
                    # inside For_i: fully pipelined; prev carries across the
                    # back-edge (first-iteration garbage write, overwritten)
                    for _rep in range(repeat_inner):
                        for b in range(BPC):
                            emit_batch(b, pipelined=True)
                else:
                    for _rep in range(repeat_inner):
                        for b in range(BPC):
                            emit_batch(b, pipelined=True)
                    # epilogue: drain the last batch's c2qT chain-major
                    # so copies/DMAs overlap the remaining chains' PE time
                    if prev[0] is not None and "c2q" not in ablate:
                        pb, pET, pqa, pcs = prev[0]
                        cqo = out_pool.tile([128, 2, LC], bf16, tag="cqo")
                        cq_r = cqT_d[pb].rearrange("a p n -> p a n", p=128)
                        for ci, (db, nh) in enumerate(CHAINS):
                            for jc in range(NJ):
                                nc.tensor.matmul(
                                    pcs[ci][:],
                                    lhsT=pqa[:, jc, db * 128:(db + 1) * 128],
                                    rhs=pET[:, jc, nh * 512:(nh + 1) * 512],
                                    start=(jc == 0), stop=(jc == NJ - 1))
                            nc.vector.tensor_copy(
                                cqo[:, db, nh * 512:(nh + 1) * 512], pcs[ci][:])
                            if ci % 2 == 1:   # d-block complete -> ship half
                                nc.scalar.dma_start(
                                    cq_r[:, db:db + 1, :], cqo[:, db:db + 1, :])
                        prev[0] = None

    nc.compile()
    return nc


def _host_prep(context_features, question_features, weight):
    import ml_dtypes
    BF = ml_dtypes.bfloat16

    c = np.ascontiguousarray(context_features, dtype=np.float32)
    q = np.ascontiguousarray(question_features, dtype=np.float32)
    w = np.asarray(weight, dtype=np.float32)[:, 0]
    wc, wq, wm = w[:D], w[D:2 * D], w[2 * D:]

    qb = (q @ wq).astype(np.float32)
    cb = c @ wc

    kT = np.ascontiguousarray((q * wm).transpose(0, 2, 1)).astype(BF)
    cT = np.ascontiguousarray(c.transpose(0, 2, 1)).astype(BF)
    qa = q.astype(BF)

    qb_t = np.ascontiguousarray(
        qb.reshape(B, NJ, 128).transpose(0, 2, 1))

    in_maps = []
    for core in range(N_CORES):
        s = slice(core * BPC, (core + 1) * BPC)
        in_maps.append({
            "kt": kT[s], "ct": cT[s], "qa": qa[s], "qb": qb_t[s],
        })
    _CACHE["cb"] = cb
    _CACHE["c"] = c
    return in_maps


def _assemble(results):
    c, cb = _CACHE["c"], _CACHE["cb"]
    cqT = np.concatenate(
        [np.asarray(r["cqt"], dtype=np.float32) for r in results], axis=0)
    red = np.concatenate([r["red"] for r in results], axis=0)  # [B,128,2,NI]
    emax = red[:, :, 0, :]
    ssum = red[:, :, 1, :]

    num = cqT.reshape(B, D, LC).transpose(0, 2, 1)
    S = ssum.transpose(0, 2, 1).reshape(B, LC)
    c2q = num / S[:, :, None]

    em = emax.transpose(0, 2, 1).reshape(B, LC)
    e2 = em * np.exp(cb)
    wgt = e2 / e2.sum(axis=1, keepdims=True)
    q2c_vec = np.einsum('bc,bcd->bd', wgt, c)
    q2c = np.broadcast_to(q2c_vec[:, None, :], (B, LC, D)).copy()
    return c2q.astype(np.float32), q2c.astype(np.float32)


def _make_runner(nc, n_cores):
    import jax
    from jax.sharding import Mesh, PartitionSpec
    from jax.experimental.shard_map import shard_map
    from concourse import mybir
    from concourse.bass2jax import (
        _bass_exec_p, install_neuronx_cc_hook, partition_id_tensor)

    install_neuronx_cc_hook()

    partition_name = nc.partition_id_tensor.name if nc.partition_id_tensor else None
    in_names, out_names, out_avals, zero_shapes = [], [], [], []
    for alloc in nc.m.functions[0].allocations:
        if not isinstance(alloc, mybir.MemoryLocationSet):
            continue
        name = alloc.memorylocations[0].name
        if alloc.kind == "ExternalInput":
            if name != partition_name:
                in_names.append(name)
        elif alloc.kind == "ExternalOutput":
            out_names.append(name)
            shape = tuple(alloc.tensor_shape)
            dtype = mybir.dt.np(alloc.dtype)
            out_avals.append(jax.core.ShapedArray(shape, dtype))
            zero_shapes.append((shape, dtype))
    n_params = len(in_names)
    all_names = list(in_names) + list(out_names)
    if partition_name is not None:
        all_names.append(partition_name)

    def _body(*args):
        operands = list(args)
        if partition_name is not None:
            operands.append(partition_id_tensor())
        outs = _bass_exec_p.bind(
            *operands,
            out_avals=tuple(out_avals),
            in_names=tuple(all_names),
            out_names=tuple(out_names),
            lowering_input_output_aliases=(),
            sim_require_finite=True,
            sim_require_nnan=True,
            nc=nc,
        )
        return tuple(outs)

    devices = jax.devices()[:n_cores]
    assert len(devices) == n_cores, f"need {n_cores} cores"
    mesh = Mesh(np.asarray(devices), ("core",))
    n_outs = len(out_names)
    fn = jax.jit(
        shard_map(
            _body, mesh=mesh,
            in_specs=(PartitionSpec("core"),) * (n_params + n_outs),
            out_specs=(PartitionSpec("core"),) * n_outs,
            check_rep=False),
        keep_unused=True,
    )
    sharding = jax.sharding.NamedSharding(mesh, PartitionSpec("core"))
    zeros = [
        jax.device_put(
            np.zeros((shape[0] * n_cores,) + tuple(shape[1:]), dtype), sharding)
        for shape, dtype in zero_shapes
    ]

    def run(in_maps):
        concat_in = [
            np.concatenate([np.asarray(m[name]) for m in in_maps], axis=0)
            for name in in_names
        ]
        dev_in = [jax.device_put(a, sharding) for a in concat_in]
        outs = fn(*dev_in, *zeros)
        results = []
        for cix in range(n_cores):
            d = {}
            for name, arr in zip(out_names, outs):
                arr = np.asarray(arr)
                per = arr.shape[0] // n_cores
                d[name] = arr[cix * per:(cix + 1) * per]
            results.append(d)
        return results

    return run


def kernel(context_features, question_features, weight):
    if "run" not in _CACHE:
        nc = build_program()
        _CACHE["nc"] = nc
        _CACHE["run"] = _make_runner(nc, N_CORES)

    in_maps = _host_prep(context_features, question_features, weight)
    results = _CACHE["run"](in_maps)
    return _assemble(results)

